# revision 47
# baseline (speedup 1.0000x reference)
"""Multi-head attention (B=1, T=1500, N=1280, H=20, D=64) on 8 NeuronCores.

Tensor-parallel by head groups, 2.5 heads/core: core c owns full heads
F0=2c, F1=2c+1 plus a window of shared head S=16+c//2 (q rows 768*(c%2),
width 768 even / 732 odd; the shared-head q projection for that window is
computed on the host and passed in as `qs`).

Kernel structure (per core):
  - projections with SBUF-image weight layouts so each weight loads in one
    big DMA; xT loads in f-half x column-chunk pieces,
  - kT/qT computed transposed (contract over d with no transposes); v in
    natural [t, d] layout with a ones column per head slot so the wv matmul
    also yields the softmax denominator,
  - exp without max subtraction (scores ~N(0,1), max < ~7, safe in fp32),
  - shared head attention runs first (overlapped with the q projection),
    then the main loop walks q-chunks: scores+exp+wv for both full heads,
    normalize, out-projection of the chunk's t-tiles, store -- so psum
    staging copies and stores spread across the whole attention phase,
  - partial outputs stored as bf16 (partials are ~1/8 of final magnitude;
    rounding adds ~0.1% vs the 2e-2 budget), halving store DMA,
  - stores issued on the gpsimd/Pool queue (SWDGE) to bypass the serial
    HWDGE descriptor device; loads on sync/SP.
The 8 partials are combined on device: psum(out + place(out2)).
"""

import os

import numpy as np

T, F, D = 1500, 1280, 64
NH = 20
QK_SCALE = D ** (-0.5)
PT = [128] * 11 + [92]          # partition tiles along t / k (sum = 1500)
PT_OFF = [128 * i for i in range(12)]
KCH_Q = [(0, 512), (512, 512), (1024, 476)]   # q-projection chunks
ECH = [512, 384, 384]           # out-proj free-dim chunks (sum = 1280)
# v block width per t-tile: bf16 v-proj matmuls run full rate at any free
# size, but f32r drops to 4 cycles/row below 256
VW = 200 if os.environ.get("KERNEL_XDT", "bf16") == "bf16" else 256
NCORES = 8
TQ = 768                        # shared-head padded q width

_CACHE = {}
LAST_RESULTS = None


class _StageDone(Exception):
    pass


def _split3(ap2d, nf, fstride):
    """Insert a middle [fstride, nf] dim into a 2D AP: iterates the 2D
    slice at column offsets fstride*f for f in range(nf)."""
    import concourse.bass as bass
    return bass.AP(
        tensor=ap2d.tensor,
        offset=ap2d.offset,
        ap=[list(ap2d.ap[0]), [fstride, nf], list(ap2d.ap[1])],
    )


def _build(mm_dtype_name):
    import concourse.bacc as bacc
    import concourse.mybir as mybir
    import concourse.tile as tile
    import concourse.bass as bass
    from contextlib import ExitStack

    use_pbc = os.environ.get("KERNEL_PBC", "1") == "1"
    use_swdge = os.environ.get("KERNEL_SWDGE", "1") == "1"
    use_split3 = os.environ.get("KERNEL_SPLIT3", "1") == "1"
    use_dummy = os.environ.get("KERNEL_DUMMY", "1") == "1"
    out_dt_f32 = os.environ.get("KERNEL_OUTDT", "bf16") == "f32"
    x_bf16 = os.environ.get("KERNEL_XDT", "bf16") == "bf16"
    stage = int(os.environ.get("KERNEL_STAGE", "9"))

    f32 = mybir.dt.float32
    bf16 = mybir.dt.bfloat16
    mm_dt = getattr(mybir.dt, mm_dtype_name)
    out_dt = f32 if out_dt_f32 else bf16
    x_dt = bf16 if x_bf16 else mm_dt

    nc = bacc.Bacc("TRN2", target_bir_lowering=False, debug=False,
                   num_devices=NCORES)

    # SBUF-image DRAM layouts: [p, blockcol] = weight[128*f + p, col]
    xT_d = nc.dram_tensor("xT", [128, 10 * T], x_dt, kind="ExternalInput").ap()
    wqk_d = nc.dram_tensor("wqk", [128, 3200], x_dt,
                           kind="ExternalInput").ap()
    bqk_d = nc.dram_tensor("bqk", [128, 3], f32, kind="ExternalInput").ap()
    wvw_d = nc.dram_tensor("wvw", [128, 10 * VW], x_dt,
                           kind="ExternalInput").ap()
    bv_d = nc.dram_tensor("bv", [128, VW], f32, kind="ExternalInput").ap()
    qs_d = nc.dram_tensor("qs", [64, TQ], mm_dt, kind="ExternalInput").ap()
    wo_d = nc.dram_tensor("wo", [192, F], mm_dt, kind="ExternalInput").ap()
    out_d = nc.dram_tensor("out", [T, F], out_dt, kind="ExternalOutput").ap()
    out2_d = nc.dram_tensor("out2", [TQ, F], out_dt,
                            kind="ExternalOutput").ap()

    EXP = mybir.ActivationFunctionType.Exp

    with tile.TileContext(nc) as tc:
        with ExitStack() as ctx:
            persist = ctx.enter_context(tc.tile_pool(name="persist", bufs=1))

            XT = persist.tile([128, 10 * T], x_dt, tag="xt", name="xt")
            WQKW = persist.tile([128, 3200], x_dt, tag="wqkw",
                                name="wqkw")
            WVW = persist.tile([128, 10 * VW], x_dt, tag="wvw", name="wvw")
            Q12 = persist.tile([128, T], mm_dt, tag="q12", name="q12")
            K12 = persist.tile([128, T], mm_dt, tag="k12", name="k12")
            K3 = persist.tile([64, T], mm_dt, tag="k3", name="k3")
            QS = persist.tile([64, TQ], mm_dt, tag="qs", name="qs")
            V = persist.tile([128, 12 * VW], mm_dt, tag="v", name="v")
            WVT12 = persist.tile([128, T], mm_dt, tag="wvt12", name="wvt12")
            WVT3 = persist.tile([64, TQ], mm_dt, tag="wvt3", name="wvt3")
            WO12 = persist.tile([128, F], mm_dt, tag="wo12", name="wo12")
            WO3 = persist.tile([64, F], mm_dt, tag="wo3", name="wo3")
            BQK = persist.tile([128, 3], f32, tag="bqk", name="bqk")
            BV = persist.tile([128, VW], f32, tag="bv", name="bv")
            # ones row at partition 64 (the Z row) for the 1/Z PE
            # broadcast; memset writes f32, then a copy rounds into the
            # matmul dtype (f32r memset is invalid ISA and fp32r matmuls
            # require rounded producers)
            ONESF = persist.tile([65, 64], f32, tag="onesf", name="onesf")
            ONES = persist.tile([65, 64], mm_dt, tag="ones", name="ones")

            # ---------------- loads (issue order = consumption order) ----
            def ld_xchunk(o, w, flo, fhi):
                if not use_split3:
                    for f in range(flo, fhi):
                        nc.sync.dma_start(
                            XT[:, T * f + o:T * f + o + w],
                            xT_d[:, T * f + o:T * f + o + w])
                    return
                nf = fhi - flo
                src = xT_d[:, T * flo + o:T * flo + o + w]
                dst = XT[:, T * flo + o:T * flo + o + w]
                nc.sync.dma_start(_split3(dst, nf, T), _split3(src, nf, T))

            # tiny memset+exp up front so the ACT table load happens while
            # the PE is still waiting on the first weight DMAs
            nc.vector.memset(ONESF[64:65, 0:64], 1.0)
            nc.vector.tensor_copy(ONES[64:65, 0:64], ONESF[64:65, 0:64])
            if use_dummy:
                dumm = persist.tile([1, 16], f32, tag="dumm", name="dumm")
                dume = persist.tile([1, 16], f32, tag="dume", name="dume")
                nc.vector.memset(dumm[0:1, 0:16], 0.0)
                nc.scalar.activation(dume[0:1, 0:16], dumm[0:1, 0:16], EXP)

            nc.sync.dma_start(WQKW[:, 1280:1920], wqk_d[:, 1280:1920])  # m1a
            ld_xchunk(0, 512, 0, 1)
            ld_xchunk(0, 512, 1, 3)
            ld_xchunk(0, 512, 3, 5)
            nc.sync.dma_start(WQKW[:, 1920:2560], wqk_d[:, 1920:2560])  # m1b
            ld_xchunk(0, 512, 5, 10)
            nc.sync.dma_start(BQK[:], bqk_d[:, :])
            nc.sync.dma_start(QS[:], qs_d[:, :])
            nc.sync.dma_start(WQKW[:, 2560:3200], wqk_d[:, 2560:3200])  # m2
            nc.sync.dma_start(WVW[:], wvw_d[:, :])
            nc.sync.dma_start(BV[:], bv_d[:, :])
            ld_xchunk(512, 512, 0, 5)
            ld_xchunk(512, 512, 5, 10)
            ld_xchunk(1024, 476, 0, 5)
            ld_xchunk(1024, 476, 5, 10)
            nc.sync.dma_start(WQKW[:, 0:1280], wqk_d[:, 0:1280])        # m0
            nc.sync.dma_start(WO3[:], wo_d[128:192, :])
            nc.sync.dma_start(WO12[:], wo_d[0:128, :])

            # ---------------- projections + shared-head attention --------
            # per k-chunk: kT (m1), shared kT (m2), v, then the shared
            # head's scores/exp/wv for the chunk's k-tiles -- the slot-2
            # attention fills PE time while xT DMAs stream in
            KCH = [(0, 512, range(0, 4)), (512, 512, range(4, 8)),
                   (1024, 476, range(8, 12))]
            S2CH = [(0, 512), (512, 256)]
            QKDEST = {0: Q12, 1: K12, 2: K3}

            attn = ExitStack()
            fin = attn.enter_context(tc.tile_pool(name="fin", bufs=6))
            epool = attn.enter_context(tc.tile_pool(name="epool", bufs=4))
            stg = attn.enter_context(tc.tile_pool(name="stg", bufs=4))

            ncopy = [0]

            def copy_eng(dst, src, mode="dve"):
                # psum->sbuf staging copies; gpsimd cannot read PSUM.
                # "dve": all DVE (ACT is exp-bound mid-loop); "alt": 1:1
                # DVE/ACT for the tail where ACT has gone idle
                if mode == "alt" and ncopy[0] % 2 == 1:
                    nc.scalar.copy(dst, src)
                else:
                    nc.vector.tensor_copy(dst, src)
                ncopy[0] += 1

            zdram = attn.enter_context(
                tc.tile_pool(name="zdram", bufs=4, space="DRAM"))

            def normalize(wvt, co, w, dst64, pbpool, pbtag,
                          lo=0, hi=66, zp=64, vlo=0, cmode="dve"):
                """wv psum tile rows lo:hi, cols co:co+w (Z row at zp, wv
                values at vlo:vlo+64) -> dst64 [64, w] = wv / Z.  1/Z is
                broadcast across partitions with a ones-column PE matmul;
                all element ops stay lane-aligned so slot 1 (lo=62) writes
                WVT12[64:128] directly with no partition-shift DMA."""
                wsb = fin.tile([128, 512], f32, tag="wsb", name="wsb")
                copy_eng(wsb[lo:hi, 0:w], wvt[lo:hi, co:co + w], cmode)
                rz = fin.tile([128, 512], mm_dt, tag="rz", name="rz")
                with nc.allow_low_precision(
                        reason="1/Z rounded to f32r for the PE broadcast"):
                    nc.vector.reciprocal(rz[zp:zp + 1, 0:w],
                                         wsb[zp:zp + 1, 0:w])
                psb = pbpool.tile([128, 512], f32, tag=pbtag, name=pbtag)
                nc.tensor.matmul(psb[vlo:vlo + 64, 0:w],
                                 ONES[zp:zp + 1, 0:64],
                                 rz[zp:zp + 1, 0:w],
                                 start=True, stop=True)
                nc.vector.tensor_mul(dst64, wsb[vlo:vlo + 64, 0:w],
                                     psb[vlo:vlo + 64, 0:w])

            with tc.tile_pool(name="pp_p", bufs=2, space="PSUM") as pp_p, \
                 tc.tile_pool(name="pp_s2", bufs=1, space="PSUM") as pp_s2, \
                 tc.tile_pool(name="pp_wv2", bufs=1, space="PSUM") as pp_wv2:
                def qkproj(m, o, w):
                    rows = 128 if m < 2 else 64
                    base, stride = (1280 * m, 128) if m < 2 else (2560, 64)
                    ps = pp_p.tile([128, 512], f32, tag="pp", name="pp")
                    for f in range(10):
                        nc.tensor.matmul(
                            ps[0:rows, 0:w],
                            WQKW[:, base + stride * f:
                                 base + stride * f + rows],
                            XT[:, T * f + o:T * f + o + w],
                            start=(f == 0), stop=(f == 9),
                        )
                    if m == 0:
                        # q-proj runs at the proj->main seam where DVE is
                        # piled up (slot-2 normalize + out2 copies); do its
                        # bias add on ACT instead
                        nc.scalar.add(QKDEST[m][0:rows, o:o + w],
                                      ps[0:rows, 0:w], BQK[0:rows, m:m + 1])
                    else:
                        nc.vector.tensor_scalar_add(
                            QKDEST[m][0:rows, o:o + w], ps[0:rows, 0:w],
                            BQK[0:rows, m:m + 1])

                wv2 = pp_wv2.tile([66, TQ], f32, tag="pwv2", name="pwv2")
                for (o, w, tts) in KCH:
                    qkproj(1, o, w)
                    qkproj(2, o, w)
                    for tt in tts:
                        pk = PT[tt]
                        ps = pp_p.tile([128, VW], f32, tag="pv", name="pv")
                        for f in range(10):
                            nc.tensor.matmul(
                                ps[0:pk, :],
                                XT[:, T * f + PT_OFF[tt]:
                                   T * f + PT_OFF[tt] + pk],
                                WVW[:, VW * f:VW * (f + 1)],
                                start=(f == 0), stop=(f == 9),
                            )
                        nc.vector.tensor_add(
                            V[0:pk, VW * tt:VW * (tt + 1)], ps[0:pk, :],
                            BV[0:pk, :])
                    for tt in (tts if stage >= 2 else []):
                        pk = PT[tt]
                        E = epool.tile([128, TQ], mm_dt, tag="E2",
                                       name="E2")
                        ps = pp_s2.tile([128, 1024], f32, tag="ps2",
                                        name="ps2")
                        for (qo, qw) in S2CH:
                            nc.tensor.matmul(
                                ps[0:pk, qo:qo + qw],
                                K3[:, PT_OFF[tt]:PT_OFF[tt] + pk],
                                QS[:, qo:qo + qw],
                                start=True, stop=True)
                        nc.scalar.activation(E[0:pk, 0:TQ],
                                             ps[0:pk, 0:TQ], EXP)
                        for (qo, qw) in S2CH:
                            nc.tensor.matmul(
                                wv2[0:66, qo:qo + qw],
                                V[0:pk, VW * tt + 132:VW * tt + 198],
                                E[0:pk, qo:qo + qw],
                                start=(tt == 0), stop=(tt == 11))
                for (o, w) in KCH_Q:
                    qkproj(0, o, w)
                for (qo, qw) in (S2CH if stage >= 2 else []):
                    normalize(wv2, qo, qw, WVT3[0:64, qo:qo + qw],
                              pp_s2, "ps2", cmode="alt")

            # ---------------- out projections + main attention ----------
            def outproj(wvt, wo, po, tsl, pk, dram, drsl, cmode="dve"):
                stage = stg.tile([128, F], out_dt, tag="stage", name="stage")
                eoff = 0
                for ec in ECH:
                    ps = po.tile([128, ECH[0]], f32, tag="po", name="po")
                    nc.tensor.matmul(ps[0:pk, 0:ec], wvt[:, tsl],
                                     wo[:, eoff:eoff + ec],
                                     start=True, stop=True)
                    copy_eng(stage[0:pk, eoff:eoff + ec], ps[0:pk, 0:ec],
                             cmode)
                    eoff += ec
                    if cmode == "alt" and eoff == ECH[0]:
                        # tail tiles: ship the first half early on the idle
                        # HWDGE queue to shorten the final drain
                        nc.sync.dma_start(dram[drsl, 0:ECH[0]],
                                          stage[0:pk, 0:ECH[0]])
                if cmode == "alt":
                    nc.sync.dma_start(dram[drsl, ECH[0]:F],
                                      stage[0:pk, ECH[0]:F])
                elif use_swdge:
                    nc.gpsimd.dma_start(dram[drsl, :], stage[0:pk, :])
                else:
                    nc.sync.dma_start(dram[drsl, :], stage[0:pk, :])

            def attn_chunk(o, w, soff, pp_sc, pp_wv, pp_bc, scw,
                           nsplits=None, cmode="dve"):
                """scores+exp+wv for both full heads over q cols o:o+w;
                slot s scores land at ps[:, soff*s:...], wv accumulates
                into a [66, 2*soff] tile; returns the wv tile."""
                # wv accumulation groups stay open across the tt loop, so
                # the two slots live in separate psum banks; slot 1 sits at
                # partition offset 62 (ones col first) so its normalized
                # output lands in WVT12[64:128] without a shift DMA
                wv01 = pp_wv.tile([128, 1024], f32, tag="pwv",
                                  name="pwv")
                for tt in range(12):
                    pk = PT[tt]
                    ps = pp_sc.tile([128, 1024], f32, tag="psc",
                                    name="psc")
                    for s in (0, 1):
                        nc.tensor.matmul(
                            ps[0:pk, soff * s:soff * s + w],
                            K12[64 * s:64 * s + 64,
                                PT_OFF[tt]:PT_OFF[tt] + pk],
                            Q12[64 * s:64 * s + 64, o:o + w],
                            start=True, stop=True)
                    E = epool.tile([128, 1024], mm_dt, tag="E", name="E")
                    if w == soff:
                        # one exp over both heads' scores
                        nc.scalar.activation(E[0:pk, 0:2 * w],
                                             ps[0:pk, 0:2 * w], EXP)
                    else:
                        # skip the unwritten hole between the slots
                        nc.scalar.activation(E[0:pk, 0:w],
                                             ps[0:pk, 0:w], EXP)
                        nc.scalar.activation(E[0:pk, soff:soff + w],
                                             ps[0:pk, soff:soff + w], EXP)
                    for s in (0, 1):
                        nc.tensor.matmul(
                            wv01[0:66, 512 * s:512 * s + w],
                            V[0:pk, VW * tt + 66 * s:VW * tt + 66 * s + 66],
                            E[0:pk, soff * s:soff * s + w],
                            start=(tt == 0), stop=(tt == 11))
                # slot 1 first: its partition-shift DMA then overlaps
                # slot 0's normalize chain; optionally normalize in column
                # pieces so the tail out-proj can start on early tiles
                for (no, nw) in (nsplits or ((0, w),)):
                    qi = slice(o + no, o + no + nw)
                    w2 = fin.tile([64, 512], mm_dt, tag="w2", name="w2")
                    normalize(wv01, 512 + no, nw, w2[0:64, 0:nw],
                              pp_bc, "po", cmode=cmode)
                    nc.sync.dma_start(WVT12[64:128, qi], w2[0:64, 0:nw])
                    normalize(wv01, no, nw, WVT12[0:64, qi], pp_bc, "po",
                              cmode=cmode)

            try:
              with tc.tile_pool(name="pp_o", bufs=2, space="PSUM") as pp_o:
                with tc.tile_pool(name="pp_sc", bufs=2,
                                  space="PSUM") as pp_sc, \
                     tc.tile_pool(name="pp_wv", bufs=1,
                                  space="PSUM") as pp_wv:
                    # shared-head out-proj: overlaps the first main chunk
                    for j in (range(6) if stage >= 3 else []):
                        outproj(WVT3, WO3, pp_o,
                                slice(128 * j, 128 * (j + 1)), 128,
                                out2_d, slice(128 * j, 128 * (j + 1)))
                    if stage < 4:
                        raise _StageDone()
                    for (o, w, tlo, thi, nsp, cm) in (
                            (0, 512, 0, 4, None, "dve"),
                            (512, 512, 4, 8, None, "dve"),
                            (1024, 476, 8, 12, ((0, 256), (256, 220)),
                             "alt")):
                        attn_chunk(o, w, 512, pp_sc, pp_wv, pp_o, "",
                                   nsplits=nsp, cmode=cm)
                        for tt in range(tlo, thi):
                            tsl = slice(PT_OFF[tt], PT_OFF[tt] + PT[tt])
                            outproj(WVT12, WO12, pp_o, tsl, PT[tt],
                                    out_d, tsl, cmode=cm)
            except _StageDone:
                pass
            attn.close()

    nc.compile()
    return nc


def _get_nc(mm_dtype_name):
    key = (mm_dtype_name,) + tuple(
        os.environ.get(k, d) for k, d in
        (("KERNEL_PBC", "1"), ("KERNEL_SWDGE", "1"), ("KERNEL_SPLIT3", "1"),
         ("KERNEL_DUMMY", "1"), ("KERNEL_OUTDT", "bf16"),
         ("KERNEL_STAGE", "9"), ("KERNEL_XDT", "bf16")))
    if key not in _CACHE:
        _CACHE[key] = _build(mm_dtype_name)
    return _CACHE[key]


def _prep_core_inputs(c, x, xT, WqT, bq, WkTs, WvT, bv, WoT):
    """Per-core inputs.  Full heads F0=2c, F1=2c+1; shared head S=16+c//2
    with q rows 768*(c%2) .. +768 (732 real for odd cores)."""
    F0, F1 = 2 * c, 2 * c + 1
    S = 16 + c // 2
    roff = TQ * (c % 2)
    ww = TQ if c % 2 == 0 else T - TQ

    def hsl(h):
        return slice(D * h, D * (h + 1))

    # SBUF-image layouts: [p, blockcol] = W[128*f + p, col]
    xim = np.ascontiguousarray(
        xT.reshape(10, 128, T).transpose(1, 0, 2).reshape(128, 10 * T))

    wqk = np.zeros((128, 3200), dtype=np.float32)
    bqk = np.zeros((128, 3), dtype=np.float32)
    blocks = [np.zeros((F, 128), dtype=np.float32) for _ in range(2)]
    blocks[0][:, 0:64] = WqT[:, hsl(F0)]
    blocks[0][:, 64:128] = WqT[:, hsl(F1)]
    blocks[1][:, 0:64] = WkTs[:, hsl(F0)]
    blocks[1][:, 64:128] = WkTs[:, hsl(F1)]
    for m in range(2):
        wqk[:, 1280 * m:1280 * (m + 1)] = (
            blocks[m].reshape(10, 128, 128).transpose(1, 0, 2)
            .reshape(128, 1280))
    wqk[:, 2560:3200] = (
        WkTs[:, hsl(S)].reshape(10, 128, 64).transpose(1, 0, 2)
        .reshape(128, 640))
    bqk[0:64, 0] = bq[hsl(F0)]
    bqk[64:128, 0] = bq[hsl(F1)]

    wvb = np.zeros((F, VW), dtype=np.float32)
    bvr = np.zeros((VW,), dtype=np.float32)
    for s, h in enumerate((F0, F1, S)):
        wvb[:, 66 * s:66 * s + 64] = WvT[:, hsl(h)]
        bvr[66 * s:66 * s + 64] = bv[hsl(h)]
        bvr[66 * s + 64] = 1.0
    wvw = np.ascontiguousarray(
        wvb.reshape(10, 128, VW).transpose(1, 0, 2).reshape(128, 10 * VW))

    qs = np.zeros((64, TQ), dtype=np.float32)
    qs[:, 0:ww] = (x[roff:roff + ww] @ WqT[:, hsl(S)] + bq[hsl(S)]).T

    wo = np.zeros((192, F), dtype=np.float32)
    wo[0:64] = WoT[hsl(F0), :]
    wo[64:128] = WoT[hsl(F1), :]
    wo[128:192] = WoT[hsl(S), :]

    if os.environ.get("KERNEL_XDT", "bf16") == "bf16":
        import ml_dtypes
        xim = xim.astype(ml_dtypes.bfloat16)
        wqk = wqk.astype(ml_dtypes.bfloat16)
        wvw = wvw.astype(ml_dtypes.bfloat16)
    return {
        "xT": xim,
        "wqk": wqk,
        "bqk": bqk,
        "wvw": wvw,
        "bv": np.broadcast_to(bvr, (128, VW)).copy(),
        "qs": qs,
        "wo": wo,
    }


def _make_runner(nc):
    """Axon-path runner (built once, reused).  Three separate jits because
    neuronx_cc_hook requires the bass module to contain only the bass_exec
    custom call: (1) on-device zero output buffers, (2) the sharded bass
    call, (3) on-device combine: psum(out + place(out2)).  Only one [T, F]
    array is transferred back; per-core uploads are cached on device."""
    import jax
    import jax.numpy as jnp
    import concourse.mybir as mybir
    from concourse import bass2jax
    from jax.experimental.shard_map import shard_map
    from jax.sharding import Mesh, PartitionSpec

    bass2jax.install_neuronx_cc_hook()

    partition_name = (nc.partition_id_tensor.name
                      if nc.partition_id_tensor else None)

    REPLICATED = {"xT"}
    in_names, out_names, out_avals, zero_templates = [], [], [], []
    for alloc in nc.m.functions[0].allocations:
        if not isinstance(alloc, mybir.MemoryLocationSet):
            continue
        name = alloc.memorylocations[0].name
        if alloc.kind == "ExternalInput":
            if name != partition_name:
                in_names.append(name)
        elif alloc.kind == "ExternalOutput":
            out_names.append(name)
            shape = tuple(alloc.tensor_shape)
            dtype = mybir.dt.np(alloc.dtype)
            out_avals.append(jax.core.ShapedArray(shape, dtype))
            zero_templates.append((shape, dtype))
    n_params = len(in_names)
    n_outs = len(out_avals)
    all_names = in_names + out_names
    if partition_name is not None:
        all_names = all_names + [partition_name]
    donate = tuple(range(n_params, n_params + n_outs))
    i_out = out_names.index("out")
    i_out2 = out_names.index("out2")

    devices = jax.devices()[:NCORES]
    mesh = Mesh(np.asarray(devices), ("core",))

    def _body(*args):
        operands = list(args)
        if partition_name is not None:
            operands.append(bass2jax.partition_id_tensor())
        outs = bass2jax._bass_exec_p.bind(
            *operands,
            out_avals=tuple(out_avals),
            in_names=tuple(all_names),
            out_names=tuple(out_names),
            lowering_input_output_aliases=(),
            sim_require_finite=False,
            sim_require_nnan=False,
            nc=nc,
        )
        return tuple(outs)

    in_specs = tuple(
        PartitionSpec() if n in REPLICATED else PartitionSpec("core")
        for n in in_names
    ) + (PartitionSpec("core"),) * n_outs
    bass_fn = jax.jit(
        shard_map(_body, mesh=mesh, in_specs=in_specs,
                  out_specs=(PartitionSpec("core"),) * n_outs,
                  check_rep=False),
        donate_argnums=donate, keep_unused=True,
    )

    def _zeros():
        return tuple(jnp.zeros(s, d) for (s, d) in zero_templates)

    zeros_fn = jax.jit(
        shard_map(_zeros, mesh=mesh, in_specs=(),
                  out_specs=(PartitionSpec("core"),) * n_outs,
                  check_rep=False))

    def _combine(o, o2):
        idx = jax.lax.axis_index("core")
        off = TQ * (idx % 2)
        z = jnp.zeros((2 * TQ, F), jnp.float32)
        z = jax.lax.dynamic_update_slice(
            z, o2.astype(jnp.float32), (off, 0))
        return jax.lax.psum(o.astype(jnp.float32) + z[:T], "core")

    reduce_fn = jax.jit(
        shard_map(_combine, mesh=mesh,
                  in_specs=(PartitionSpec("core"), PartitionSpec("core")),
                  out_specs=PartitionSpec(), check_rep=False))

    dev_cache = {}

    def run(in_maps):
        args = []
        for n in in_names:
            if n in REPLICATED:
                arr = np.asarray(in_maps[0][n])
            else:
                arr = np.concatenate(
                    [np.asarray(in_maps[c][n]) for c in range(NCORES)],
                    axis=0)
            fp = (arr.shape, hash(arr.tobytes()))
            cached = dev_cache.get(n)
            if cached is not None and cached[0] == fp:
                args.append(cached[1])
            else:
                dev_arr = jax.device_put(
                    arr, jax.sharding.NamedSharding(
                        mesh,
                        PartitionSpec() if n in REPLICATED
                        else PartitionSpec("core")))
                dev_cache[n] = (fp, dev_arr)
                args.append(dev_arr)
        zeros = zeros_fn()
        outs = bass_fn(*args, *zeros)
        total = reduce_fn(outs[i_out], outs[i_out2])
        return np.asarray(total)

    return run


def kernel(x, Wq, bq, Wk, Wv, bv, Wo, bo):
    global LAST_RESULTS

    mm_dtype_name = os.environ.get("KERNEL_MM_DTYPE", "float32r")
    nc = _get_nc(mm_dtype_name)

    x = np.asarray(x, dtype=np.float32).reshape(T, F)
    xT = np.ascontiguousarray(x.T)
    WqT = np.ascontiguousarray(np.asarray(Wq, dtype=np.float32).T)
    WkTs = (np.ascontiguousarray(np.asarray(Wk, dtype=np.float32).T)
            * np.float32(QK_SCALE))
    WvT = np.ascontiguousarray(np.asarray(Wv, dtype=np.float32).T)
    WoT = np.ascontiguousarray(np.asarray(Wo, dtype=np.float32).T)
    bq = np.asarray(bq, dtype=np.float32)
    bvv = np.asarray(bv, dtype=np.float32)

    in_maps = [
        _prep_core_inputs(c, x, xT, WqT, bq, WkTs, WvT, bvv, WoT)
        for c in range(NCORES)
    ]

    from concourse._compat import axon_active

    if axon_active():
        key = (mm_dtype_name, "runner") + tuple(
            os.environ.get(k, d) for k, d in
            (("KERNEL_PBC", "1"), ("KERNEL_SWDGE", "1"),
             ("KERNEL_SPLIT3", "1"), ("KERNEL_DUMMY", "1"),
             ("KERNEL_OUTDT", "bf16"), ("KERNEL_XDT", "bf16")))
        if key not in _CACHE:
            _CACHE[key] = _make_runner(nc)
        out = np.array(_CACHE[key](in_maps), dtype=np.float32)
    else:
        from concourse.bass_utils import run_bass_kernel_spmd
        trace = os.environ.get("KERNEL_TRACE", "0") == "1"
        res = run_bass_kernel_spmd(nc, in_maps, core_ids=list(range(NCORES)),
                                   trace=trace)
        LAST_RESULTS = res
        out = np.zeros((T, F), dtype=np.float32)
        for c in range(NCORES):
            out += res.results[c]["out"].astype(np.float32)
            roff = TQ * (c % 2)
            ww = min(TQ, T - roff)
            out[roff:roff + ww] += (
                res.results[c]["out2"][0:ww].astype(np.float32))
    out += np.asarray(bo, dtype=np.float32)
    return out.reshape(1, T, F)


# revision 48
# speedup vs baseline: 1.0311x; 1.0311x over previous
"""Multi-head attention (B=1, T=1500, N=1280, H=20, D=64) on 8 NeuronCores.

Tensor-parallel by head groups, 2.5 heads/core: core c owns full heads
F0=2c, F1=2c+1 plus a window of shared head S=16+c//2 (q rows 768*(c%2),
width 768 even / 732 odd; the shared-head q projection for that window is
computed on the host and passed in as `qs`).

Kernel structure (per core):
  - projections with SBUF-image weight layouts so each weight loads in one
    big DMA; xT loads in f-half x column-chunk pieces,
  - kT/qT computed transposed (contract over d with no transposes); v in
    natural [t, d] layout with a ones column per head slot so the wv matmul
    also yields the softmax denominator,
  - exp without max subtraction (scores ~N(0,1), max < ~7, safe in fp32),
  - shared head attention runs first (overlapped with the q projection),
    then the main loop walks q-chunks: scores+exp+wv for both full heads,
    normalize, out-projection of the chunk's t-tiles, store -- so psum
    staging copies and stores spread across the whole attention phase,
  - partial outputs stored as bf16 (partials are ~1/8 of final magnitude;
    rounding adds ~0.1% vs the 2e-2 budget), halving store DMA,
  - stores issued on the gpsimd/Pool queue (SWDGE) to bypass the serial
    HWDGE descriptor device; loads on sync/SP.
The 8 partials are combined on device: psum(out + place(out2)).
"""

import os

import numpy as np

T, F, D = 1500, 1280, 64
NH = 20
QK_SCALE = D ** (-0.5)
PT = [128] * 11 + [92]          # partition tiles along t / k (sum = 1500)
PT_OFF = [128 * i for i in range(12)]
KCH_Q = [(0, 512), (512, 512), (1024, 476)]   # q-projection chunks
ECH = [512, 384, 384]           # out-proj free-dim chunks (sum = 1280)
# v block width per t-tile: bf16 v-proj matmuls run full rate at any free
# size, but f32r drops to 4 cycles/row below 256
VW = 200 if os.environ.get("KERNEL_XDT", "bf16") == "bf16" else 256
NCORES = 8
TQ = 768                        # shared-head padded q width

_CACHE = {}
LAST_RESULTS = None


class _StageDone(Exception):
    pass


def _split3(ap2d, nf, fstride):
    """Insert a middle [fstride, nf] dim into a 2D AP: iterates the 2D
    slice at column offsets fstride*f for f in range(nf)."""
    import concourse.bass as bass
    return bass.AP(
        tensor=ap2d.tensor,
        offset=ap2d.offset,
        ap=[list(ap2d.ap[0]), [fstride, nf], list(ap2d.ap[1])],
    )


def _build(mm_dtype_name):
    import concourse.bacc as bacc
    import concourse.mybir as mybir
    import concourse.tile as tile
    import concourse.bass as bass
    from contextlib import ExitStack

    use_pbc = os.environ.get("KERNEL_PBC", "1") == "1"
    use_swdge = os.environ.get("KERNEL_SWDGE", "1") == "1"
    use_split3 = os.environ.get("KERNEL_SPLIT3", "1") == "1"
    use_dummy = os.environ.get("KERNEL_DUMMY", "1") == "1"
    out_dt_f32 = os.environ.get("KERNEL_OUTDT", "bf16") == "f32"
    x_bf16 = os.environ.get("KERNEL_XDT", "bf16") == "bf16"
    stage = int(os.environ.get("KERNEL_STAGE", "9"))

    f32 = mybir.dt.float32
    bf16 = mybir.dt.bfloat16
    mm_dt = getattr(mybir.dt, mm_dtype_name)
    out_dt = f32 if out_dt_f32 else bf16
    x_dt = bf16 if x_bf16 else mm_dt

    nc = bacc.Bacc("TRN2", target_bir_lowering=False, debug=False,
                   num_devices=NCORES)

    # SBUF-image DRAM layouts: [p, blockcol] = weight[128*f + p, col]
    xT_d = nc.dram_tensor("xT", [128, 10 * T], x_dt, kind="ExternalInput").ap()
    wqk_d = nc.dram_tensor("wqk", [128, 3200], x_dt,
                           kind="ExternalInput").ap()
    bqk_d = nc.dram_tensor("bqk", [128, 3], f32, kind="ExternalInput").ap()
    wvw_d = nc.dram_tensor("wvw", [128, 10 * VW], x_dt,
                           kind="ExternalInput").ap()
    bv_d = nc.dram_tensor("bv", [128, VW], f32, kind="ExternalInput").ap()
    qs_d = nc.dram_tensor("qs", [64, TQ], mm_dt, kind="ExternalInput").ap()
    wo_d = nc.dram_tensor("wo", [192, F], mm_dt, kind="ExternalInput").ap()
    out_d = nc.dram_tensor("out", [T, F], out_dt, kind="ExternalOutput").ap()
    out2_d = nc.dram_tensor("out2", [TQ, F], out_dt,
                            kind="ExternalOutput").ap()

    EXP = mybir.ActivationFunctionType.Exp

    with tile.TileContext(nc) as tc:
        with ExitStack() as ctx:
            persist = ctx.enter_context(tc.tile_pool(name="persist", bufs=1))

            XT = persist.tile([128, 10 * T], x_dt, tag="xt", name="xt")
            WQKW = persist.tile([128, 3200], x_dt, tag="wqkw",
                                name="wqkw")
            WVW = persist.tile([128, 10 * VW], x_dt, tag="wvw", name="wvw")
            Q12 = persist.tile([128, T], mm_dt, tag="q12", name="q12")
            K12 = persist.tile([128, T], mm_dt, tag="k12", name="k12")
            K3 = persist.tile([64, T], mm_dt, tag="k3", name="k3")
            QS = persist.tile([64, TQ], mm_dt, tag="qs", name="qs")
            V = persist.tile([128, 12 * VW], mm_dt, tag="v", name="v")
            WVT12 = persist.tile([128, T], mm_dt, tag="wvt12", name="wvt12")
            WVT3 = persist.tile([64, TQ], mm_dt, tag="wvt3", name="wvt3")
            WO12 = persist.tile([128, F], mm_dt, tag="wo12", name="wo12")
            WO3 = persist.tile([64, F], mm_dt, tag="wo3", name="wo3")
            BQK = persist.tile([128, 3], f32, tag="bqk", name="bqk")
            BV = persist.tile([128, VW], f32, tag="bv", name="bv")
            # ones row at partition 64 (the Z row) for the 1/Z PE
            # broadcast; memset writes f32, then a copy rounds into the
            # matmul dtype (f32r memset is invalid ISA and fp32r matmuls
            # require rounded producers)
            ONESF = persist.tile([65, 64], f32, tag="onesf", name="onesf")
            ONES = persist.tile([65, 64], mm_dt, tag="ones", name="ones")

            # ---------------- loads (issue order = consumption order) ----
            def ld_xchunk(o, w, flo, fhi):
                if not use_split3:
                    for f in range(flo, fhi):
                        nc.sync.dma_start(
                            XT[:, T * f + o:T * f + o + w],
                            xT_d[:, T * f + o:T * f + o + w])
                    return
                nf = fhi - flo
                src = xT_d[:, T * flo + o:T * flo + o + w]
                dst = XT[:, T * flo + o:T * flo + o + w]
                nc.sync.dma_start(_split3(dst, nf, T), _split3(src, nf, T))

            # tiny memset+exp up front so the ACT table load happens while
            # the PE is still waiting on the first weight DMAs
            nc.vector.memset(ONESF[64:65, 0:64], 1.0)
            nc.vector.tensor_copy(ONES[64:65, 0:64], ONESF[64:65, 0:64])
            if use_dummy:
                dumm = persist.tile([1, 16], f32, tag="dumm", name="dumm")
                dume = persist.tile([1, 16], f32, tag="dume", name="dume")
                nc.vector.memset(dumm[0:1, 0:16], 0.0)
                nc.scalar.activation(dume[0:1, 0:16], dumm[0:1, 0:16], EXP)

            nc.sync.dma_start(WQKW[:, 1280:1920], wqk_d[:, 1280:1920])  # m1a
            ld_xchunk(0, 512, 0, 1)
            ld_xchunk(0, 512, 1, 3)
            ld_xchunk(0, 512, 3, 5)
            nc.sync.dma_start(WQKW[:, 1920:2560], wqk_d[:, 1920:2560])  # m1b
            ld_xchunk(0, 512, 5, 10)
            nc.sync.dma_start(BQK[:], bqk_d[:, :])
            nc.sync.dma_start(QS[:], qs_d[:, :])
            nc.sync.dma_start(WQKW[:, 2560:3200], wqk_d[:, 2560:3200])  # m2
            nc.sync.dma_start(WVW[:], wvw_d[:, :])
            nc.sync.dma_start(BV[:], bv_d[:, :])
            ld_xchunk(512, 512, 0, 5)
            ld_xchunk(512, 512, 5, 10)
            ld_xchunk(1024, 476, 0, 5)
            ld_xchunk(1024, 476, 5, 10)
            nc.sync.dma_start(WQKW[:, 0:1280], wqk_d[:, 0:1280])        # m0
            nc.sync.dma_start(WO3[:], wo_d[128:192, :])
            nc.sync.dma_start(WO12[:], wo_d[0:128, :])

            # ---------------- projections + shared-head attention --------
            # per k-chunk: kT (m1), shared kT (m2), v, then the shared
            # head's scores/exp/wv for the chunk's k-tiles -- the slot-2
            # attention fills PE time while xT DMAs stream in
            KCH = [(0, 512, range(0, 4)), (512, 512, range(4, 8)),
                   (1024, 476, range(8, 12))]
            S2CH = [(0, 512), (512, 256)]
            QKDEST = {0: Q12, 1: K12, 2: K3}

            attn = ExitStack()
            fin = attn.enter_context(tc.tile_pool(name="fin", bufs=6))
            epool = attn.enter_context(tc.tile_pool(name="epool", bufs=4))
            stg = attn.enter_context(tc.tile_pool(name="stg", bufs=4))

            ncopy = [0]

            def copy_eng(dst, src, mode="dve"):
                # psum->sbuf staging copies; gpsimd cannot read PSUM.
                # "dve": all DVE (ACT is exp-bound mid-loop); "alt": 1:1
                # DVE/ACT for the tail where ACT has gone idle
                if mode == "alt" and ncopy[0] % 2 == 1:
                    nc.scalar.copy(dst, src)
                else:
                    nc.vector.tensor_copy(dst, src)
                ncopy[0] += 1

            zdram = attn.enter_context(
                tc.tile_pool(name="zdram", bufs=4, space="DRAM"))

            def normalize(wvt, co, w, dst64, pbpool, pbtag,
                          lo=0, hi=66, zp=64, vlo=0, cmode="dve"):
                """wv psum tile rows lo:hi, cols co:co+w (Z row at zp, wv
                values at vlo:vlo+64) -> dst64 [64, w] = wv / Z.  1/Z is
                broadcast across partitions with a ones-column PE matmul;
                all element ops stay lane-aligned so slot 1 (lo=62) writes
                WVT12[64:128] directly with no partition-shift DMA."""
                wsb = fin.tile([128, 512], f32, tag="wsb", name="wsb")
                copy_eng(wsb[lo:hi, 0:w], wvt[lo:hi, co:co + w], cmode)
                rz = fin.tile([128, 512], mm_dt, tag="rz", name="rz")
                with nc.allow_low_precision(
                        reason="1/Z rounded to f32r for the PE broadcast"):
                    nc.vector.reciprocal(rz[zp:zp + 1, 0:w],
                                         wsb[zp:zp + 1, 0:w])
                psb = pbpool.tile([128, 512], f32, tag=pbtag, name=pbtag)
                nc.tensor.matmul(psb[vlo:vlo + 64, 0:w],
                                 ONES[zp:zp + 1, 0:64],
                                 rz[zp:zp + 1, 0:w],
                                 start=True, stop=True)
                nc.vector.tensor_mul(dst64, wsb[vlo:vlo + 64, 0:w],
                                     psb[vlo:vlo + 64, 0:w])

            with tc.tile_pool(name="pp_p", bufs=2, space="PSUM") as pp_p, \
                 tc.tile_pool(name="pp_s2", bufs=1, space="PSUM") as pp_s2, \
                 tc.tile_pool(name="pp_wv2", bufs=1, space="PSUM") as pp_wv2:
                def qkproj(m, o, w):
                    rows = 128 if m < 2 else 64
                    base, stride = (1280 * m, 128) if m < 2 else (2560, 64)
                    ps = pp_p.tile([128, 512], f32, tag="pp", name="pp")
                    for f in range(10):
                        nc.tensor.matmul(
                            ps[0:rows, 0:w],
                            WQKW[:, base + stride * f:
                                 base + stride * f + rows],
                            XT[:, T * f + o:T * f + o + w],
                            start=(f == 0), stop=(f == 9),
                        )
                    if m == 0:
                        # q-proj runs at the proj->main seam where DVE is
                        # piled up (slot-2 normalize + out2 copies); do its
                        # bias add on ACT instead
                        nc.scalar.add(QKDEST[m][0:rows, o:o + w],
                                      ps[0:rows, 0:w], BQK[0:rows, m:m + 1])
                    else:
                        nc.vector.tensor_scalar_add(
                            QKDEST[m][0:rows, o:o + w], ps[0:rows, 0:w],
                            BQK[0:rows, m:m + 1])

                wv2 = pp_wv2.tile([66, TQ], f32, tag="pwv2", name="pwv2")
                for (o, w, tts) in KCH:
                    qkproj(1, o, w)
                    qkproj(2, o, w)
                    for tt in tts:
                        pk = PT[tt]
                        ps = pp_p.tile([128, VW], f32, tag="pv", name="pv")
                        for f in range(10):
                            nc.tensor.matmul(
                                ps[0:pk, :],
                                XT[:, T * f + PT_OFF[tt]:
                                   T * f + PT_OFF[tt] + pk],
                                WVW[:, VW * f:VW * (f + 1)],
                                start=(f == 0), stop=(f == 9),
                            )
                        nc.vector.tensor_add(
                            V[0:pk, VW * tt:VW * (tt + 1)], ps[0:pk, :],
                            BV[0:pk, :])
                    for tt in (tts if stage >= 2 else []):
                        pk = PT[tt]
                        E = epool.tile([128, TQ], mm_dt, tag="E2",
                                       name="E2")
                        ps = pp_s2.tile([128, 1024], f32, tag="ps2",
                                        name="ps2")
                        for (qo, qw) in S2CH:
                            nc.tensor.matmul(
                                ps[0:pk, qo:qo + qw],
                                K3[:, PT_OFF[tt]:PT_OFF[tt] + pk],
                                QS[:, qo:qo + qw],
                                start=True, stop=True)
                        nc.scalar.activation(E[0:pk, 0:TQ],
                                             ps[0:pk, 0:TQ], EXP)
                        for (qo, qw) in S2CH:
                            nc.tensor.matmul(
                                wv2[0:66, qo:qo + qw],
                                V[0:pk, VW * tt + 132:VW * tt + 198],
                                E[0:pk, qo:qo + qw],
                                start=(tt == 0), stop=(tt == 11))
                for (o, w) in KCH_Q:
                    qkproj(0, o, w)
                for (qo, qw) in (S2CH if stage >= 2 else []):
                    normalize(wv2, qo, qw, WVT3[0:64, qo:qo + qw],
                              pp_s2, "ps2", cmode="alt")

            # ---------------- out projections + main attention ----------
            def outproj(wvt, wo, po, tsl, pk, dram, drsl, cmode="dve"):
                stage = stg.tile([128, F], out_dt, tag="stage", name="stage")
                eoff = 0
                for ec in ECH:
                    ps = po.tile([128, ECH[0]], f32, tag="po", name="po")
                    nc.tensor.matmul(ps[0:pk, 0:ec], wvt[:, tsl],
                                     wo[:, eoff:eoff + ec],
                                     start=True, stop=True)
                    copy_eng(stage[0:pk, eoff:eoff + ec], ps[0:pk, 0:ec],
                             cmode)
                    eoff += ec
                    if cmode == "alt" and eoff == ECH[0]:
                        # tail tiles: ship the first half early on the idle
                        # HWDGE queue to shorten the final drain
                        nc.sync.dma_start(dram[drsl, 0:ECH[0]],
                                          stage[0:pk, 0:ECH[0]])
                if cmode == "alt":
                    nc.sync.dma_start(dram[drsl, ECH[0]:F],
                                      stage[0:pk, ECH[0]:F])
                elif use_swdge:
                    nc.gpsimd.dma_start(dram[drsl, :], stage[0:pk, :])
                else:
                    nc.sync.dma_start(dram[drsl, :], stage[0:pk, :])

            def attn_chunk(o, w, soff, pp_sc, pp_wv, pp_bc, scw,
                           nsplits=None, cmode="dve"):
                """scores+exp+wv for both full heads over q cols o:o+w;
                slot s scores land at ps[:, soff*s:...], wv accumulates
                into a [66, 2*soff] tile; returns the wv tile."""
                # wv accumulation groups stay open across the tt loop, so
                # the two slots live in separate psum banks; slot 1 sits at
                # partition offset 62 (ones col first) so its normalized
                # output lands in WVT12[64:128] without a shift DMA
                wv01 = pp_wv.tile([128, 1024], f32, tag="pwv",
                                  name="pwv")
                for tt in range(12):
                    pk = PT[tt]
                    ps = pp_sc.tile([128, 1024], f32, tag="psc",
                                    name="psc")
                    for s in (0, 1):
                        nc.tensor.matmul(
                            ps[0:pk, soff * s:soff * s + w],
                            K12[64 * s:64 * s + 64,
                                PT_OFF[tt]:PT_OFF[tt] + pk],
                            Q12[64 * s:64 * s + 64, o:o + w],
                            start=True, stop=True)
                    E = epool.tile([128, 1024], mm_dt, tag="E", name="E")
                    # one exp over both heads' scores; for w < soff the
                    # 36 hole cols hold stale finite psum -- junk exp
                    # values there are never read downstream
                    nc.scalar.activation(E[0:pk, 0:soff + w],
                                         ps[0:pk, 0:soff + w], EXP)
                    for s in (0, 1):
                        nc.tensor.matmul(
                            wv01[0:66, 512 * s:512 * s + w],
                            V[0:pk, VW * tt + 66 * s:VW * tt + 66 * s + 66],
                            E[0:pk, soff * s:soff * s + w],
                            start=(tt == 0), stop=(tt == 11))
                # slot 1 first: its partition-shift DMA then overlaps
                # slot 0's normalize chain; optionally normalize in column
                # pieces so the tail out-proj can start on early tiles
                for (no, nw) in (nsplits or ((0, w),)):
                    qi = slice(o + no, o + no + nw)
                    w2 = fin.tile([64, 512], mm_dt, tag="w2", name="w2")
                    normalize(wv01, 512 + no, nw, w2[0:64, 0:nw],
                              pp_bc, "po", cmode=cmode)
                    nc.sync.dma_start(WVT12[64:128, qi], w2[0:64, 0:nw])
                    normalize(wv01, no, nw, WVT12[0:64, qi], pp_bc, "po",
                              cmode=cmode)

            try:
              with tc.tile_pool(name="pp_o", bufs=2, space="PSUM") as pp_o:
                with tc.tile_pool(name="pp_sc", bufs=2,
                                  space="PSUM") as pp_sc, \
                     tc.tile_pool(name="pp_wv", bufs=1,
                                  space="PSUM") as pp_wv:
                    # shared-head out-proj: overlaps the first main chunk
                    for j in (range(6) if stage >= 3 else []):
                        outproj(WVT3, WO3, pp_o,
                                slice(128 * j, 128 * (j + 1)), 128,
                                out2_d, slice(128 * j, 128 * (j + 1)))
                    if stage < 4:
                        raise _StageDone()
                    for (o, w, tlo, thi, nsp, cm) in (
                            (0, 512, 0, 4, None, "dve"),
                            (512, 512, 4, 8, None, "dve"),
                            (1024, 476, 8, 12, ((0, 256), (256, 220)),
                             "alt")):
                        attn_chunk(o, w, 512, pp_sc, pp_wv, pp_o, "",
                                   nsplits=nsp, cmode=cm)
                        for tt in range(tlo, thi):
                            tsl = slice(PT_OFF[tt], PT_OFF[tt] + PT[tt])
                            outproj(WVT12, WO12, pp_o, tsl, PT[tt],
                                    out_d, tsl, cmode=cm)
            except _StageDone:
                pass
            attn.close()

    nc.compile()
    return nc


def _get_nc(mm_dtype_name):
    key = (mm_dtype_name,) + tuple(
        os.environ.get(k, d) for k, d in
        (("KERNEL_PBC", "1"), ("KERNEL_SWDGE", "1"), ("KERNEL_SPLIT3", "1"),
         ("KERNEL_DUMMY", "1"), ("KERNEL_OUTDT", "bf16"),
         ("KERNEL_STAGE", "9"), ("KERNEL_XDT", "bf16")))
    if key not in _CACHE:
        _CACHE[key] = _build(mm_dtype_name)
    return _CACHE[key]


def _prep_core_inputs(c, x, xT, WqT, bq, WkTs, WvT, bv, WoT):
    """Per-core inputs.  Full heads F0=2c, F1=2c+1; shared head S=16+c//2
    with q rows 768*(c%2) .. +768 (732 real for odd cores)."""
    F0, F1 = 2 * c, 2 * c + 1
    S = 16 + c // 2
    roff = TQ * (c % 2)
    ww = TQ if c % 2 == 0 else T - TQ

    def hsl(h):
        return slice(D * h, D * (h + 1))

    # SBUF-image layouts: [p, blockcol] = W[128*f + p, col]
    xim = np.ascontiguousarray(
        xT.reshape(10, 128, T).transpose(1, 0, 2).reshape(128, 10 * T))

    wqk = np.zeros((128, 3200), dtype=np.float32)
    bqk = np.zeros((128, 3), dtype=np.float32)
    blocks = [np.zeros((F, 128), dtype=np.float32) for _ in range(2)]
    blocks[0][:, 0:64] = WqT[:, hsl(F0)]
    blocks[0][:, 64:128] = WqT[:, hsl(F1)]
    blocks[1][:, 0:64] = WkTs[:, hsl(F0)]
    blocks[1][:, 64:128] = WkTs[:, hsl(F1)]
    for m in range(2):
        wqk[:, 1280 * m:1280 * (m + 1)] = (
            blocks[m].reshape(10, 128, 128).transpose(1, 0, 2)
            .reshape(128, 1280))
    wqk[:, 2560:3200] = (
        WkTs[:, hsl(S)].reshape(10, 128, 64).transpose(1, 0, 2)
        .reshape(128, 640))
    bqk[0:64, 0] = bq[hsl(F0)]
    bqk[64:128, 0] = bq[hsl(F1)]

    wvb = np.zeros((F, VW), dtype=np.float32)
    bvr = np.zeros((VW,), dtype=np.float32)
    for s, h in enumerate((F0, F1, S)):
        wvb[:, 66 * s:66 * s + 64] = WvT[:, hsl(h)]
        bvr[66 * s:66 * s + 64] = bv[hsl(h)]
        bvr[66 * s + 64] = 1.0
    wvw = np.ascontiguousarray(
        wvb.reshape(10, 128, VW).transpose(1, 0, 2).reshape(128, 10 * VW))

    qs = np.zeros((64, TQ), dtype=np.float32)
    qs[:, 0:ww] = (x[roff:roff + ww] @ WqT[:, hsl(S)] + bq[hsl(S)]).T

    wo = np.zeros((192, F), dtype=np.float32)
    wo[0:64] = WoT[hsl(F0), :]
    wo[64:128] = WoT[hsl(F1), :]
    wo[128:192] = WoT[hsl(S), :]

    if os.environ.get("KERNEL_XDT", "bf16") == "bf16":
        import ml_dtypes
        xim = xim.astype(ml_dtypes.bfloat16)
        wqk = wqk.astype(ml_dtypes.bfloat16)
        wvw = wvw.astype(ml_dtypes.bfloat16)
    return {
        "xT": xim,
        "wqk": wqk,
        "bqk": bqk,
        "wvw": wvw,
        "bv": np.broadcast_to(bvr, (128, VW)).copy(),
        "qs": qs,
        "wo": wo,
    }


def _make_runner(nc):
    """Axon-path runner (built once, reused).  Three separate jits because
    neuronx_cc_hook requires the bass module to contain only the bass_exec
    custom call: (1) on-device zero output buffers, (2) the sharded bass
    call, (3) on-device combine: psum(out + place(out2)).  Only one [T, F]
    array is transferred back; per-core uploads are cached on device."""
    import jax
    import jax.numpy as jnp
    import concourse.mybir as mybir
    from concourse import bass2jax
    from jax.experimental.shard_map import shard_map
    from jax.sharding import Mesh, PartitionSpec

    bass2jax.install_neuronx_cc_hook()

    partition_name = (nc.partition_id_tensor.name
                      if nc.partition_id_tensor else None)

    REPLICATED = {"xT"}
    in_names, out_names, out_avals, zero_templates = [], [], [], []
    for alloc in nc.m.functions[0].allocations:
        if not isinstance(alloc, mybir.MemoryLocationSet):
            continue
        name = alloc.memorylocations[0].name
        if alloc.kind == "ExternalInput":
            if name != partition_name:
                in_names.append(name)
        elif alloc.kind == "ExternalOutput":
            out_names.append(name)
            shape = tuple(alloc.tensor_shape)
            dtype = mybir.dt.np(alloc.dtype)
            out_avals.append(jax.core.ShapedArray(shape, dtype))
            zero_templates.append((shape, dtype))
    n_params = len(in_names)
    n_outs = len(out_avals)
    all_names = in_names + out_names
    if partition_name is not None:
        all_names = all_names + [partition_name]
    donate = tuple(range(n_params, n_params + n_outs))
    i_out = out_names.index("out")
    i_out2 = out_names.index("out2")

    devices = jax.devices()[:NCORES]
    mesh = Mesh(np.asarray(devices), ("core",))

    def _body(*args):
        operands = list(args)
        if partition_name is not None:
            operands.append(bass2jax.partition_id_tensor())
        outs = bass2jax._bass_exec_p.bind(
            *operands,
            out_avals=tuple(out_avals),
            in_names=tuple(all_names),
            out_names=tuple(out_names),
            lowering_input_output_aliases=(),
            sim_require_finite=False,
            sim_require_nnan=False,
            nc=nc,
        )
        return tuple(outs)

    in_specs = tuple(
        PartitionSpec() if n in REPLICATED else PartitionSpec("core")
        for n in in_names
    ) + (PartitionSpec("core"),) * n_outs
    bass_fn = jax.jit(
        shard_map(_body, mesh=mesh, in_specs=in_specs,
                  out_specs=(PartitionSpec("core"),) * n_outs,
                  check_rep=False),
        donate_argnums=donate, keep_unused=True,
    )

    def _zeros():
        return tuple(jnp.zeros(s, d) for (s, d) in zero_templates)

    zeros_fn = jax.jit(
        shard_map(_zeros, mesh=mesh, in_specs=(),
                  out_specs=(PartitionSpec("core"),) * n_outs,
                  check_rep=False))

    def _combine(o, o2):
        idx = jax.lax.axis_index("core")
        off = TQ * (idx % 2)
        z = jnp.zeros((2 * TQ, F), jnp.float32)
        z = jax.lax.dynamic_update_slice(
            z, o2.astype(jnp.float32), (off, 0))
        return jax.lax.psum(o.astype(jnp.float32) + z[:T], "core")

    reduce_fn = jax.jit(
        shard_map(_combine, mesh=mesh,
                  in_specs=(PartitionSpec("core"), PartitionSpec("core")),
                  out_specs=PartitionSpec(), check_rep=False))

    dev_cache = {}

    def run(in_maps):
        args = []
        for n in in_names:
            if n in REPLICATED:
                arr = np.asarray(in_maps[0][n])
            else:
                arr = np.concatenate(
                    [np.asarray(in_maps[c][n]) for c in range(NCORES)],
                    axis=0)
            fp = (arr.shape, hash(arr.tobytes()))
            cached = dev_cache.get(n)
            if cached is not None and cached[0] == fp:
                args.append(cached[1])
            else:
                dev_arr = jax.device_put(
                    arr, jax.sharding.NamedSharding(
                        mesh,
                        PartitionSpec() if n in REPLICATED
                        else PartitionSpec("core")))
                dev_cache[n] = (fp, dev_arr)
                args.append(dev_arr)
        zeros = zeros_fn()
        outs = bass_fn(*args, *zeros)
        total = reduce_fn(outs[i_out], outs[i_out2])
        return np.asarray(total)

    return run


def kernel(x, Wq, bq, Wk, Wv, bv, Wo, bo):
    global LAST_RESULTS

    mm_dtype_name = os.environ.get("KERNEL_MM_DTYPE", "float32r")
    nc = _get_nc(mm_dtype_name)

    x = np.asarray(x, dtype=np.float32).reshape(T, F)
    xT = np.ascontiguousarray(x.T)
    WqT = np.ascontiguousarray(np.asarray(Wq, dtype=np.float32).T)
    WkTs = (np.ascontiguousarray(np.asarray(Wk, dtype=np.float32).T)
            * np.float32(QK_SCALE))
    WvT = np.ascontiguousarray(np.asarray(Wv, dtype=np.float32).T)
    WoT = np.ascontiguousarray(np.asarray(Wo, dtype=np.float32).T)
    bq = np.asarray(bq, dtype=np.float32)
    bvv = np.asarray(bv, dtype=np.float32)

    in_maps = [
        _prep_core_inputs(c, x, xT, WqT, bq, WkTs, WvT, bvv, WoT)
        for c in range(NCORES)
    ]

    from concourse._compat import axon_active

    if axon_active():
        key = (mm_dtype_name, "runner") + tuple(
            os.environ.get(k, d) for k, d in
            (("KERNEL_PBC", "1"), ("KERNEL_SWDGE", "1"),
             ("KERNEL_SPLIT3", "1"), ("KERNEL_DUMMY", "1"),
             ("KERNEL_OUTDT", "bf16"), ("KERNEL_XDT", "bf16")))
        if key not in _CACHE:
            _CACHE[key] = _make_runner(nc)
        out = np.array(_CACHE[key](in_maps), dtype=np.float32)
    else:
        from concourse.bass_utils import run_bass_kernel_spmd
        trace = os.environ.get("KERNEL_TRACE", "0") == "1"
        res = run_bass_kernel_spmd(nc, in_maps, core_ids=list(range(NCORES)),
                                   trace=trace)
        LAST_RESULTS = res
        out = np.zeros((T, F), dtype=np.float32)
        for c in range(NCORES):
            out += res.results[c]["out"].astype(np.float32)
            roff = TQ * (c % 2)
            ww = min(TQ, T - roff)
            out[roff:roff + ww] += (
                res.results[c]["out2"][0:ww].astype(np.float32))
    out += np.asarray(bo, dtype=np.float32)
    return out.reshape(1, T, F)
